# revision 4
# baseline (speedup 1.0000x reference)
"""AttnCutLoss Trainium2 kernel (v2).

Reference math (B=4096 rows, S=4096 positions, f1 metric, tau=0.95):
    tp    = cumsum(labels, axis=1)
    r     = 2*tp / (k + total)          [exact algebraic form of the f1 weight,
                                         incl. the tp==0 / total==0 guards]
    q     = exp(r/tau); norm = sum_j q; w = 1/norm
    loss  = -sum(log(output)*w)/B = -(1/B) * sum_rows [ (sum_j log(output)) / norm ]

Approximations (tolerance is rel 2e-2; these land ~1e-3 combined):
  * labels are pre-folded 8x on host: lab8[i] = sum of labels[8i..8i+7].
    cumsum(lab8) gives tp at k=8,16,...,4096 EXACTLY; norm ~= 8*sum_i f(8i).
    f = exp(2.105*tp/(T+k)) spans only [1, ~1.43] so the right-endpoint
    Riemann sum bias is ~+3e-4 relative. 8x fewer scan/recip/mult/exp elems.
  * output is sent as fp16 scaled by 32 (values in [0.032, 32): no fp16
    subnormals even after pairwise products; ln rel err ~2^-12).
  * log-sum pass is halved by pairwise products: sum ln(x) = sum ln(x_lo*x_hi)
    with x_lo/x_hi the two contiguous halves of the row -> DVE 2x fp16
    tensor_tensor mode; products in [1e-3, 1024) stay fp16-normal. Host
    subtracts the exact 2048*ln(32*32) scale correction per row.

Per-core engine split (512 rows/core, 4 groups of 128 partitions):
  DVE : cumsum scan [128,512]; reciprocal_approx_fast(d); fold u*v [128,2048]
  GPS : d = k_tile + T (per-partition scalar add); r = tp * inv (stt)
  ACT : Exp(r*2/tau) with accum_out -> norms; Ln(fold) with accum_out -> logsums
  PE  : idle
  DMA : output fp16 1MB/group + folded labels 0.5MB/core  (~4.5MB/core vs 12MB)
Host: loss = -(sum over rows (logsum_row - LNCORR)/(8*normacc_row))/B.
"""

import numpy as np
import ml_dtypes

B = 4096
S = 4096
TAU = 0.95
NCORES = 8
RPC = B // NCORES          # rows per core = 512
G = RPC // 128             # row groups per core = 4
F = 8                      # host fold factor for labels
SF = S // F                # folded row length = 512
OSCALE = 32.0              # host scale on output before fp16 cast
LNCORR = (S // 2) * float(np.log(OSCALE * OSCALE))  # per-row logsum correction

_PROGRAM_CACHE = {}


def _build_program(repeats: int = 1):
    import concourse.bass as bass
    import concourse.tile as tile
    import concourse.mybir as mybir
    from concourse import bacc
    from contextlib import ExitStack
    import contextlib

    dt = mybir.dt
    alu = mybir.AluOpType
    act = mybir.ActivationFunctionType

    nc = bacc.Bacc("TRN2")
    outh = nc.dram_tensor("outh", [RPC, S], dt.float16, kind="ExternalInput")
    lab8 = nc.dram_tensor("lab8", [128, G * SF], dt.float16, kind="ExternalInput")
    kt = nc.dram_tensor("kt", [128, SF], dt.float32, kind="ExternalInput")
    norms = nc.dram_tensor("norms", [128, G], dt.float32, kind="ExternalOutput")
    logsums = nc.dram_tensor("logsums", [128, G], dt.float32, kind="ExternalOutput")

    with ExitStack() as ctx:
        tc = ctx.enter_context(tile.TileContext(nc))
        consts = ctx.enter_context(tc.tile_pool(name="consts", bufs=1))
        labp = ctx.enter_context(tc.tile_pool(name="labp", bufs=1))
        outp = ctx.enter_context(tc.tile_pool(name="outp", bufs=3))
        tpp = ctx.enter_context(tc.tile_pool(name="tpp", bufs=2))
        dp = ctx.enter_context(tc.tile_pool(name="dp", bufs=2))
        invp = ctx.enter_context(tc.tile_pool(name="invp", bufs=2))
        rp = ctx.enter_context(tc.tile_pool(name="rp", bufs=2))
        foldp = ctx.enter_context(tc.tile_pool(name="foldp", bufs=2))
        dump = ctx.enter_context(tc.tile_pool(name="dump", bufs=1))
        accp = ctx.enter_context(tc.tile_pool(name="accp", bufs=1))

        kt_sb = consts.tile([128, SF], dt.float32)
        nc.sync.dma_start(kt_sb[:, :], kt[:, :])

        naccs_sb = accp.tile([128, G], dt.float32)
        logsums_sb = accp.tile([128, G], dt.float32)
        qdump = dump.tile([128, SF], dt.bfloat16)
        ldump = dump.tile([128, S // 2], dt.bfloat16)

        loop_cm = tc.For_i(0, repeats, 1) if repeats > 1 else contextlib.nullcontext()
        with loop_cm:
            lab_t = labp.tile([128, G * SF], dt.float16, tag="lab")
            nc.sync.dma_start(lab_t[:, :], lab8[:, :])
            for g in range(G):
                out_t = outp.tile([128, S], dt.float16, tag="outv")
                nc.sync.dma_start(out_t[:, :], outh[g * 128:(g + 1) * 128, :])

                # tp = cumsum(lab8) along free dim; exact integers in fp16
                tp_t = tpp.tile([128, SF], dt.float32, tag="tp")
                nc.vector.tensor_tensor_scan(
                    tp_t[:, :], lab_t[:, g * SF:(g + 1) * SF],
                    lab_t[:, g * SF:(g + 1) * SF], 0.0, alu.add, alu.bypass
                )

                # d = k + T  (T = tp[:, -1], per-partition scalar) on GPSIMD
                d_t = dp.tile([128, SF], dt.float32, tag="d")
                nc.gpsimd.tensor_scalar_add(d_t[:, :], kt_sb[:, :],
                                            tp_t[:, SF - 1:SF])

                # inv = 1/d on DVE (approx, ~51 ULP)
                inv_t = invp.tile([128, SF], dt.float32, tag="inv")
                nc.vector.reciprocal_approx_fast(out=inv_t[:, :], in_=d_t[:, :])

                # fold: prod = out[:, :S/2] * out[:, S/2:]  (fp16 2x TT mode)
                fold_t = foldp.tile([128, S // 2], dt.float16, tag="fold")
                nc.vector.tensor_tensor(
                    fold_t[:, :], out_t[:, :S // 2], out_t[:, S // 2:], alu.mult
                )

                # r = tp * inv on GPSIMD
                r_t = rp.tile([128, SF], dt.float32, tag="r")
                nc.gpsimd.tensor_tensor(
                    r_t[:, :], tp_t[:, :], inv_t[:, :], alu.mult
                )

                # logsum accumulation: Ln over folded products
                nc.scalar.activation(
                    ldump[:, :], fold_t[:, :], act.Ln,
                    accum_out=logsums_sb[:, g:g + 1],
                )
                # norm accumulation: Exp(r * 2/tau)
                nc.scalar.activation(
                    qdump[:, :], r_t[:, :], act.Exp,
                    scale=2.0 / TAU,
                    accum_out=naccs_sb[:, g:g + 1],
                )

        nc.sync.dma_start(norms[:, :], naccs_sb[:, :])
        nc.sync.dma_start(logsums[:, :], logsums_sb[:, :])

    nc.finalize()
    return nc


def _make_consts():
    k = (np.arange(1, SF + 1, dtype=np.float32) * F)  # 8, 16, ..., 4096
    kt = np.ascontiguousarray(np.broadcast_to(k, (128, SF))).astype(np.float32)
    return kt


def _prep_inputs(output, labels):
    """Host-side shard + dtype/layout prep. Returns per-core in_maps."""
    output = np.asarray(output)
    labels = np.asarray(labels)
    assert output.shape == (B, S, 1) and labels.shape == (B, S)

    outh_full = (output.reshape(B, S).astype(np.float32, copy=False) * OSCALE
                 ).astype(np.float16)
    # fold labels 8x: integer counts 0..8, exact in fp16
    lab8_full = labels.reshape(B, SF, F).sum(axis=2, dtype=np.float32
                                             ).astype(np.float16)

    kt = _make_consts()
    in_maps = []
    for c in range(NCORES):
        sl = slice(c * RPC, (c + 1) * RPC)
        # lab8 layout: [128 partitions, G*SF]; col-block g = rows g*128..g*128+127
        lab8_c = np.ascontiguousarray(
            lab8_full[sl].reshape(G, 128, SF).transpose(1, 0, 2).reshape(128, G * SF))
        in_maps.append({
            "outh": np.ascontiguousarray(outh_full[sl]),
            "lab8": lab8_c,
            "kt": kt,
        })
    return in_maps


def _postprocess(res):
    total = 0.0
    for c in range(NCORES):
        naccs = np.asarray(res.results[c]["norms"], dtype=np.float64)
        logs = np.asarray(res.results[c]["logsums"], dtype=np.float64)
        total += float(np.sum((logs - LNCORR) / (F * naccs)))
    return np.float32(-total / B)


def _run(output, labels, trace=False):
    from concourse.bass_utils import run_bass_kernel_spmd

    if "prog" not in _PROGRAM_CACHE:
        _PROGRAM_CACHE["prog"] = _build_program()
    nc = _PROGRAM_CACHE["prog"]

    in_maps = _prep_inputs(output, labels)
    res = run_bass_kernel_spmd(nc, in_maps, core_ids=list(range(NCORES)),
                               trace=trace)
    return _postprocess(res), res


def kernel(output, labels):
    loss, _ = _run(output, labels, trace=False)
    return loss


# revision 7
# speedup vs baseline: 6.6330x; 6.6330x over previous
"""AttnCutLoss Trainium2 kernel (v2).

Reference math (B=4096 rows, S=4096 positions, f1 metric, tau=0.95):
    tp    = cumsum(labels, axis=1)
    r     = 2*tp / (k + total)          [exact algebraic form of the f1 weight,
                                         incl. the tp==0 / total==0 guards]
    q     = exp(r/tau); norm = sum_j q; w = 1/norm
    loss  = -sum(log(output)*w)/B = -(1/B) * sum_rows [ (sum_j log(output)) / norm ]

Approximations (tolerance is rel 2e-2; these land ~1e-3 combined):
  * labels are pre-folded 8x on host: lab8[i] = sum of labels[8i..8i+7].
    cumsum(lab8) gives tp at k=8,16,...,4096 EXACTLY; norm ~= 8*sum_i f(8i).
    f = exp(2.105*tp/(T+k)) spans only [1, ~1.43] so the right-endpoint
    Riemann sum bias is ~+3e-4 relative. 8x fewer scan/recip/mult/exp elems.
  * output is sent as fp16 scaled by 32 (values in [0.032, 32): no fp16
    subnormals even after pairwise products; ln rel err ~2^-12).
  * log-sum pass is halved by pairwise products: sum ln(x) = sum ln(x_lo*x_hi)
    with x_lo/x_hi the two contiguous halves of the row -> DVE 2x fp16
    tensor_tensor mode; products in [1e-3, 1024) stay fp16-normal. Host
    subtracts the exact 2048*ln(32*32) scale correction per row.

Per-core engine split (512 rows/core, 4 groups of 128 partitions):
  DVE : cumsum scan [128,512]; reciprocal_approx_fast(d); fold u*v [128,2048]
  GPS : d = k_tile + T (per-partition scalar add); r = tp * inv (stt)
  ACT : Exp(r*2/tau) with accum_out -> norms; Ln(fold) with accum_out -> logsums
  PE  : idle
  DMA : output fp16 1MB/group + folded labels 0.5MB/core  (~4.5MB/core vs 12MB)
Host: loss = -(sum over rows (logsum_row - LNCORR)/(8*normacc_row))/B.
"""

import numpy as np
import ml_dtypes

B = 4096
S = 4096
TAU = 0.95
NCORES = 8
RPC = B // NCORES          # rows per core = 512
G = RPC // 128             # row groups per core = 4
F = 8                      # host fold factor for labels
SF = S // F                # folded row length = 512
OSCALE = 32.0              # host scale on output before fp16 cast
LNCORR = (S // 2) * float(np.log(OSCALE * OSCALE))  # per-row logsum correction

_PROGRAM_CACHE = {}


def _build_program(repeats: int = 1):
    import concourse.bass as bass
    import concourse.tile as tile
    import concourse.mybir as mybir
    from concourse import bacc
    from contextlib import ExitStack
    import contextlib

    dt = mybir.dt
    alu = mybir.AluOpType
    act = mybir.ActivationFunctionType

    nc = bacc.Bacc("TRN2")
    outh = nc.dram_tensor("outh", [RPC, S], dt.float16, kind="ExternalInput")
    lab8 = nc.dram_tensor("lab8", [128, G * SF], dt.float16, kind="ExternalInput")
    kt = nc.dram_tensor("kt", [128, SF], dt.float32, kind="ExternalInput")
    norms = nc.dram_tensor("norms", [128, G], dt.float32, kind="ExternalOutput")
    logsums = nc.dram_tensor("logsums", [128, G], dt.float32, kind="ExternalOutput")

    with ExitStack() as ctx:
        tc = ctx.enter_context(tile.TileContext(nc))
        consts = ctx.enter_context(tc.tile_pool(name="consts", bufs=1))
        labp = ctx.enter_context(tc.tile_pool(name="labp", bufs=1))
        outp = ctx.enter_context(tc.tile_pool(name="outp", bufs=3))
        tpp = ctx.enter_context(tc.tile_pool(name="tpp", bufs=2))
        dp = ctx.enter_context(tc.tile_pool(name="dp", bufs=2))
        invp = ctx.enter_context(tc.tile_pool(name="invp", bufs=2))
        rp = ctx.enter_context(tc.tile_pool(name="rp", bufs=4))
        foldp = ctx.enter_context(tc.tile_pool(name="foldp", bufs=4))
        dump = ctx.enter_context(tc.tile_pool(name="dump", bufs=1))
        accp = ctx.enter_context(tc.tile_pool(name="accp", bufs=1))

        # Pre-load ACT table set 6 (natural_log_exp_and_others): serves BOTH
        # Ln and Exp, so the act-table-load pass inserts no in-loop reloads
        # (greedy per-func choice would otherwise reload on every Ln<->Exp
        # switch, ~1.3us each).
        _li = mybir.InstLoadActFuncSet(
            name=nc.get_next_instruction_name(), ins=[], outs=[])
        _li.act_func_set_id = 6
        nc.scalar.add_instruction(_li)

        kt_sb = consts.tile([128, SF], dt.float32)
        nc.sync.dma_start(kt_sb[:, :], kt[:, :])

        naccs_sb = accp.tile([128, G], dt.float32)
        logsums_sb = accp.tile([128, G], dt.float32)
        qdump = dump.tile([128, SF], dt.bfloat16)
        ldump = dump.tile([128, S // 2], dt.bfloat16)

        loop_cm = tc.For_i(0, repeats, 1) if repeats > 1 else contextlib.nullcontext()
        with loop_cm:
            lab_t = labp.tile([128, G * SF], dt.float16, tag="lab")
            nc.sync.dma_start(lab_t[:, :], lab8[:, :])
            fold_ts = []
            r_ts = []
            for g in range(G):
                out_t = outp.tile([128, S], dt.float16, tag="outv")
                nc.sync.dma_start(out_t[:, :], outh[g * 128:(g + 1) * 128, :])

                # tp = cumsum(lab8) along free dim; exact integers
                tp_t = tpp.tile([128, SF], dt.float32, tag="tp")
                nc.vector.tensor_tensor_scan(
                    tp_t[:, :], lab_t[:, g * SF:(g + 1) * SF],
                    lab_t[:, g * SF:(g + 1) * SF], 0.0, alu.add, alu.bypass
                )

                # d = k + T  (T = tp[:, -1], per-partition scalar) on GPSIMD
                d_t = dp.tile([128, SF], dt.float32, tag="d")
                nc.gpsimd.tensor_scalar_add(d_t[:, :], kt_sb[:, :],
                                            tp_t[:, SF - 1:SF])

                # inv = 1/d on DVE (approx, ~51 ULP)
                inv_t = invp.tile([128, SF], dt.float32, tag="inv")
                nc.vector.reciprocal_approx_fast(out=inv_t[:, :], in_=d_t[:, :])

                # fold: prod = out[:, :S/2] * out[:, S/2:]  (fp16 2x TT mode)
                fold_t = foldp.tile([128, S // 2], dt.float16, tag="fold")
                nc.vector.tensor_tensor(
                    fold_t[:, :], out_t[:, :S // 2], out_t[:, S // 2:], alu.mult
                )
                fold_ts.append(fold_t)

                # r = tp * inv on GPSIMD
                r_t = rp.tile([128, SF], dt.float32, tag="r")
                nc.gpsimd.tensor_tensor(
                    r_t[:, :], tp_t[:, :], inv_t[:, :], alu.mult
                )
                r_ts.append(r_t)

            # ACT phase: batch all Ln then all Exp -> 2 table loads/iter not 6
            for g in range(G):
                nc.scalar.activation(
                    ldump[:, :], fold_ts[g][:, :], act.Ln,
                    accum_out=logsums_sb[:, g:g + 1],
                )
            for g in range(G):
                nc.scalar.activation(
                    qdump[:, :], r_ts[g][:, :], act.Exp,
                    scale=2.0 / TAU,
                    accum_out=naccs_sb[:, g:g + 1],
                )

        nc.sync.dma_start(norms[:, :], naccs_sb[:, :])
        nc.sync.dma_start(logsums[:, :], logsums_sb[:, :])

    nc.finalize()
    return nc


def _make_consts():
    k = (np.arange(1, SF + 1, dtype=np.float32) * F)  # 8, 16, ..., 4096
    kt = np.ascontiguousarray(np.broadcast_to(k, (128, SF))).astype(np.float32)
    return kt


def _prep_inputs(output, labels):
    """Host-side shard + dtype/layout prep. Returns per-core in_maps."""
    output = np.asarray(output)
    labels = np.asarray(labels)
    assert output.shape == (B, S, 1) and labels.shape == (B, S)

    outh_full = (output.reshape(B, S).astype(np.float32, copy=False) * OSCALE
                 ).astype(np.float16)
    # fold labels 8x: integer counts 0..8, exact in fp16
    lab8_full = labels.reshape(B, SF, F).sum(axis=2, dtype=np.float32
                                             ).astype(np.float16)

    kt = _make_consts()
    in_maps = []
    for c in range(NCORES):
        sl = slice(c * RPC, (c + 1) * RPC)
        # lab8 layout: [128 partitions, G*SF]; col-block g = rows g*128..g*128+127
        lab8_c = np.ascontiguousarray(
            lab8_full[sl].reshape(G, 128, SF).transpose(1, 0, 2).reshape(128, G * SF))
        in_maps.append({
            "outh": np.ascontiguousarray(outh_full[sl]),
            "lab8": lab8_c,
            "kt": kt,
        })
    return in_maps


def _postprocess(res):
    total = 0.0
    for c in range(NCORES):
        naccs = np.asarray(res.results[c]["norms"], dtype=np.float64)
        logs = np.asarray(res.results[c]["logsums"], dtype=np.float64)
        total += float(np.sum((logs - LNCORR) / (F * naccs)))
    return np.float32(-total / B)


def _run(output, labels, trace=False):
    from concourse.bass_utils import run_bass_kernel_spmd

    if "prog" not in _PROGRAM_CACHE:
        _PROGRAM_CACHE["prog"] = _build_program()
    nc = _PROGRAM_CACHE["prog"]

    in_maps = _prep_inputs(output, labels)
    res = run_bass_kernel_spmd(nc, in_maps, core_ids=list(range(NCORES)),
                               trace=trace)
    return _postprocess(res), res


def kernel(output, labels):
    loss, _ = _run(output, labels, trace=False)
    return loss


# revision 21
# speedup vs baseline: 12.7916x; 1.9285x over previous
"""AttnCutLoss Trainium2 kernel (v3).

Reference math (B=4096 rows, S=4096 positions, f1 metric, tau=0.95):
    tp    = cumsum(labels, axis=1)
    r     = 2*tp / (k + total)          [exact algebraic form of the f1 weight,
                                         incl. the tp==0 / total==0 guards]
    q     = exp(r/tau); norm = sum_j q; w = 1/norm
    loss  = -sum(log(output)*w)/B = -(1/B) * sum_rows [ (sum_j log(output)) / norm ]

Approximations (tolerance is rel 2e-2; these land ~1e-3 combined):
  * labels are pre-folded Fx on host (F=16): lab16[i] = sum of a 16-block.
    cumsum gives tp at k=16,32,...,4096 EXACTLY; norm ~= 16*sum_i f(16i).
    f = exp(2.105*tp/(T+k)) spans only [1, ~1.43] so the right-endpoint
    Riemann bias is ~+6e-4 relative. 16x fewer scan/recip/mult/exp elems.
  * output is sent as fp16 scaled by 32 (no fp16 subnormals after the first
    pairwise product; ln rel err ~2^-12).
  * log-sum pass is quartered by two pairwise-product folds:
    sum ln(x) = sum ln((a*b)*(c*d)); fold1 fp16 (DVE 2x TT mode),
    fold2 f32 (products up to 32^4 overflow fp16). Host subtracts the exact
    S*ln(32) scale correction per row.

Per-core engine split (512 rows/core, 4 groups of 128 partitions):
  DVE : scan; reciprocal_approx_fast; r=tp*inv; fold1; fold2 (or half)
  ACT : d = k+T via Identity(bias=T); Ln(fold2) accum; Exp(r*2/tau) accum
        (single act-table set 6 serves Ln+Exp: no in-loop table reloads)
  POOL: optional half of fold2 (plain TensorTensor only; TensorScalarPtr on
        Pool costs ~6.5us/op on HW - never use it there)
  DMA : output fp16 4MB/core + labels 0.25MB/core, optionally split across
        the SP and ACT HWDGE rings
Host: loss = -(sum over rows (logsum_row - S*ln 32)/(F*normacc_row))/B.
"""

import numpy as np
import ml_dtypes

B = 4096
S = 4096
TAU = 0.95
NCORES = 8
RPC = B // NCORES          # rows per core = 512
G = RPC // 128             # row groups per core = 4
F = 16                     # host fold factor for labels
SF = S // F                # folded row length = 256
OSCALE = 32.0              # host scale on output before fp16 cast
LNCORR = S * float(np.log(OSCALE))  # per-row logsum correction

_PROGRAM_CACHE = {}


def _build_program(repeats: int = 1, d_eng: str = "act", r_eng: str = "dve",
                   dma_only: bool = False, static_dma: bool = False,
                   fold2: bool = True, pool_fold2: bool = False,
                   split_rings: bool = True, dma_pack: int = 1):
    import concourse.bass as bass
    import concourse.tile as tile
    import concourse.mybir as mybir
    from concourse import bacc
    from contextlib import ExitStack
    import contextlib

    dt = mybir.dt
    alu = mybir.AluOpType
    act = mybir.ActivationFunctionType

    nc = bacc.Bacc("TRN2")
    # output groups packed side-by-side: [128, G*S] fp16
    outh = nc.dram_tensor("outh", [128, G * S], dt.float16, kind="ExternalInput")
    lab8 = nc.dram_tensor("lab8", [128, G * SF], dt.float16, kind="ExternalInput")
    kt = nc.dram_tensor("kt", [128, SF], dt.float32, kind="ExternalInput")
    norms = nc.dram_tensor("norms", [128, G], dt.float32, kind="ExternalOutput")
    logsums = nc.dram_tensor("logsums", [128, G], dt.float32, kind="ExternalOutput")

    HF = S // 2            # fold1 width
    QF = S // 4            # fold2 width

    with ExitStack() as ctx:
        tc = ctx.enter_context(tile.TileContext(nc))
        consts = ctx.enter_context(tc.tile_pool(name="consts", bufs=1))
        labp = ctx.enter_context(tc.tile_pool(name="labp", bufs=1))
        outp = ctx.enter_context(tc.tile_pool(name="outp", bufs=4 if static_dma else 3))
        tpp = ctx.enter_context(tc.tile_pool(name="tpp", bufs=2))
        dp = ctx.enter_context(tc.tile_pool(name="dp", bufs=2))
        invp = ctx.enter_context(tc.tile_pool(name="invp", bufs=2))
        rp = ctx.enter_context(tc.tile_pool(name="rp", bufs=4))
        foldp = ctx.enter_context(tc.tile_pool(name="foldp", bufs=2))
        fold2p = ctx.enter_context(tc.tile_pool(name="fold2p", bufs=4))
        dump = ctx.enter_context(tc.tile_pool(name="dump", bufs=1))
        accp = ctx.enter_context(tc.tile_pool(name="accp", bufs=1))

        # Pre-load ACT table set 6 (natural_log_exp_and_others): serves BOTH
        # Ln and Exp, so the act-table-load pass inserts no in-loop reloads.
        _li = mybir.InstLoadActFuncSet(
            name=nc.get_next_instruction_name(), ins=[], outs=[])
        _li.act_func_set_id = 6
        nc.scalar.add_instruction(_li)

        kt_sb = consts.tile([128, SF], dt.float32)
        nc.sync.dma_start(kt_sb[:, :], kt[:, :])

        naccs_sb = accp.tile([128, G], dt.float32)
        logsums_sb = accp.tile([128, G], dt.float32)
        qdump = dump.tile([128, SF], dt.bfloat16)
        ldump = dump.tile([128, QF if fold2 else HF], dt.bfloat16)

        def out_dma(g, tile_t):
            eng = nc.scalar if (split_rings and g % 2 == 1) else nc.sync
            eng.dma_start(tile_t[:, :], outh[:, g * S:(g + 1) * S])

        static_outs = []
        if static_dma:
            lab_t = labp.tile([128, G * SF], dt.float16, tag="lab")
            nc.sync.dma_start(lab_t[:, :], lab8[:, :])
            for g in range(G):
                sout = outp.tile([128, S], dt.float16, tag="outv")
                out_dma(g, sout)
                static_outs.append(sout)

        loop_cm = tc.For_i(0, repeats, 1) if repeats > 1 else contextlib.nullcontext()
        with loop_cm:
            if not static_dma:
                lab_t = labp.tile([128, G * SF], dt.float16, tag="lab")
                nc.sync.dma_start(lab_t[:, :], lab8[:, :])
            if dma_only:
                assert dma_pack in (1, 2, 4)
                npk = G // dma_pack
                for i in range(npk):
                    out_t = outp.tile([128, S * dma_pack], dt.float16, tag="outv")
                    eng = nc.scalar if (split_rings and i % 2 == 1) else nc.sync
                    eng.dma_start(out_t[:, :],
                                  outh[:, i * S * dma_pack:(i + 1) * S * dma_pack])
            fold_ts = []
            r_ts = []
            for g in range(G if not dma_only else 0):
                if static_dma:
                    out_t = static_outs[g]
                else:
                    out_t = outp.tile([128, S], dt.float16, tag="outv")
                    out_dma(g, out_t)

                # tp = cumsum(lab) along free dim; exact integers
                tp_t = tpp.tile([128, SF], dt.float32, tag="tp")
                nc.vector.tensor_tensor_scan(
                    tp_t[:, :], lab_t[:, g * SF:(g + 1) * SF],
                    lab_t[:, g * SF:(g + 1) * SF], 0.0, alu.add, alu.bypass
                )

                # d = k + T  (T = tp[:, -1], per-partition scalar)
                d_t = dp.tile([128, SF], dt.float32, tag="d")
                if d_eng == "act":
                    nc.scalar.activation(
                        d_t[:, :], kt_sb[:, :], act.Identity,
                        bias=tp_t[:, SF - 1:SF], scale=1.0)
                else:
                    deng = nc.gpsimd if d_eng == "pool" else nc.vector
                    deng.tensor_scalar_add(d_t[:, :], kt_sb[:, :],
                                           tp_t[:, SF - 1:SF])

                # inv = 1/d on DVE (approx, ~51 ULP)
                inv_t = invp.tile([128, SF], dt.float32, tag="inv")
                nc.vector.reciprocal_approx_fast(out=inv_t[:, :], in_=d_t[:, :])

                # fold1: prod = out[:, :HF] * out[:, HF:]  (fp16 2x TT mode)
                fold_t = foldp.tile([128, HF], dt.float16, tag="fold")
                nc.vector.tensor_tensor(
                    fold_t[:, :], out_t[:, :HF], out_t[:, HF:], alu.mult
                )

                if fold2:
                    f2_t = fold2p.tile([128, QF], dt.float32, tag="fold2")
                    if pool_fold2:
                        h = QF // 2
                        nc.vector.tensor_tensor(
                            f2_t[:, :h], fold_t[:, :h],
                            fold_t[:, QF:QF + h], alu.mult)
                        nc.gpsimd.tensor_tensor(
                            f2_t[:, h:], fold_t[:, h:QF],
                            fold_t[:, QF + h:], alu.mult)
                    else:
                        nc.vector.tensor_tensor(
                            f2_t[:, :], fold_t[:, :QF], fold_t[:, QF:], alu.mult)
                    fold_ts.append(f2_t)
                else:
                    fold_ts.append(fold_t)

                # r = tp * inv
                r_t = rp.tile([128, SF], dt.float32, tag="r")
                reng = nc.gpsimd if r_eng == "pool" else nc.vector
                reng.tensor_tensor(
                    r_t[:, :], tp_t[:, :], inv_t[:, :], alu.mult
                )
                r_ts.append(r_t)

            # ACT phase: batch all Ln then all Exp
            for g in range(G if not dma_only else 0):
                nc.scalar.activation(
                    ldump[:, :], fold_ts[g][:, :], act.Ln,
                    accum_out=logsums_sb[:, g:g + 1],
                )
            for g in range(G if not dma_only else 0):
                nc.scalar.activation(
                    qdump[:, :], r_ts[g][:, :], act.Exp,
                    scale=2.0 / TAU,
                    accum_out=naccs_sb[:, g:g + 1],
                )

        if not dma_only:
            nc.sync.dma_start(norms[:, :], naccs_sb[:, :])
            nc.sync.dma_start(logsums[:, :], logsums_sb[:, :])

    nc.finalize()
    return nc


def _make_consts():
    k = (np.arange(1, SF + 1, dtype=np.float32) * F)  # 16, 32, ..., 4096
    kt = np.ascontiguousarray(np.broadcast_to(k, (128, SF))).astype(np.float32)
    return kt


def _prep_inputs(output, labels):
    """Host-side shard + dtype/layout prep. Returns per-core in_maps."""
    output = np.asarray(output)
    labels = np.asarray(labels)
    assert output.shape == (B, S, 1) and labels.shape == (B, S)

    outh_full = (output.reshape(B, S).astype(np.float32, copy=False) * OSCALE
                 ).astype(np.float16)
    # fold labels Fx: integer counts 0..F, exact in fp16
    lab8_full = labels.reshape(B, SF, F).sum(axis=2, dtype=np.float32
                                             ).astype(np.float16)

    kt = _make_consts()
    in_maps = []
    for c in range(NCORES):
        sl = slice(c * RPC, (c + 1) * RPC)
        # layout: [128 partitions, G*W]; col-block g = rows g*128..g*128+127
        outh_c = np.ascontiguousarray(
            outh_full[sl].reshape(G, 128, S).transpose(1, 0, 2).reshape(128, G * S))
        lab8_c = np.ascontiguousarray(
            lab8_full[sl].reshape(G, 128, SF).transpose(1, 0, 2).reshape(128, G * SF))
        in_maps.append({
            "outh": outh_c,
            "lab8": lab8_c,
            "kt": kt,
        })
    return in_maps


def _postprocess(res):
    total = 0.0
    for c in range(NCORES):
        naccs = np.asarray(res.results[c]["norms"], dtype=np.float64)
        logs = np.asarray(res.results[c]["logsums"], dtype=np.float64)
        total += float(np.sum((logs - LNCORR) / (F * naccs)))
    return np.float32(-total / B)


def _run(output, labels, trace=False):
    from concourse.bass_utils import run_bass_kernel_spmd

    if "prog" not in _PROGRAM_CACHE:
        _PROGRAM_CACHE["prog"] = _build_program()
    nc = _PROGRAM_CACHE["prog"]

    in_maps = _prep_inputs(output, labels)
    res = run_bass_kernel_spmd(nc, in_maps, core_ids=list(range(NCORES)),
                               trace=trace)
    return _postprocess(res), res


def kernel(output, labels):
    loss, _ = _run(output, labels, trace=False)
    return loss
